# revision 12
# baseline (speedup 1.0000x reference)
"""Trainium2 Bass kernel for nn_MixMLP (moe_routing) — fp8 DoubleRow, v6.

Strategy (device = pure big-layer GEMM; everything else host-side):
  - Output is binary: y_hard + y_soft - stop_grad(y_soft) == y_hard numerically,
    so each edge decision is  (logit0 - logit1) + (gum0 - gum1) >= 0.
  - Only the DIFFERENCE of adjacent final-layer columns matters:
        d = h3 @ wd,  wd = w3[:, 0::2] - w3[:, 1::2]   (1024 x 32640)
    decision = (d + bdd - gd) >= 0,  gd = gum1 - gum0, bdd = b3[0::2]-b3[1::2].
  - h3 (the 3-layer MLP on x, [512, 1024]) is computed EXACTLY on host
    (0.5 GFLOP of BLAS) and shipped as fp8 BETA*h3 — the on-device MLP in
    v4/v5 cost ~10us of serial PE head for 0.7% of the FLOPs.
  - Rows are routed by mask = x[:,0] > 0. Host sorts rows big-first and
    assigns each 128-row chunk to ONE expert, forcing an even 2+2 chunk
    split so every core runs exactly 2 slots (v4/v5 let the boundary chunk
    force nslots=3 == +50% device matmuls). The <=|b-256| misrouted rows
    (9 for the graded seed) get d recomputed exactly on host.
  - 8 cores = 2 row-groups (one per 2-chunk group) x 4 column-quarters of
    wd. Device streams wd in 16 x 512-col fp8 chunks and does 128
    DoubleRow matmuls (N=512) per core, chunk-major so the PE chases the
    DMA stream with no barrier; psum evacuated by ACT/DVE alternating
    into 2048-col output windows DMA'd on the scalar ring.
  - Host: dec = (t >= 0), t = d + bdd - gd; near-ties |t| < 7e-3+0.05|d|
    recomputed exactly in float64, then scattered into the symmetric
    adjacency. Result is exact regardless of device matmul precision.
"""

import os
import numpy as np
import ml_dtypes

import concourse.bass as bass
import concourse.mybir as mybir
import concourse.tile as tile
from concourse.tile import add_dep_helper
from concourse import bacc
from concourse.bass_utils import run_bass_kernel_spmd

B = 512
COND = 64
N_NODES = 256
E = 32640  # upper-tri edges
NCORES = 8
QCOLS = E // 4  # 8160 columns of wd per core
QP = 8192  # padded to multiple of 2048
NSLOTS = 2
R = NSLOTS * 128

ALPHA = 512.0  # wd scale into fp8
BETA = 32.0  # h3 scale into fp8
SCALE = ALPHA * BETA  # big-layer psum holds SCALE*d
SCALE_OUT = 512.0  # output tensor holds SCALE_OUT*d

F32 = mybir.dt.float32
FP8 = mybir.dt.float8e4
NP_FP8 = ml_dtypes.float8_e4m3

# |t| < TOL_ABS + TOL_REL*|d| edges are recomputed exactly on host
TOL_ABS = 7.0e-3
TOL_REL = 0.05  # covers fp8 quantization of h3/wd/output

WIN = 2048  # output window width (4 chunks)

_program_cache = {}
last_results = None  # BassKernelResults of the most recent device run


def build_program():
    """One SPMD program: 256 rows (2 slots), one expert, one wd quarter."""
    DR = mybir.MatmulPerfMode.DoubleRow
    nc = bacc.Bacc(None, target_bir_lowering=False)

    h3q = nc.dram_tensor("h3q", [128, 8, R], FP8, kind="ExternalInput")  # BETA*h3
    # wd pre-packed on host per 512-col chunk, [c, p, ko, n] — each chunk DMA
    # reads 4KB contiguous per partition (the v6 [1024, QP] layout made the
    # DMA gather 512-byte strided runs and capped the stream at ~220 GB/s)
    wdq = nc.dram_tensor("wdq", [16, 128, 8, 512], FP8, kind="ExternalInput")
    dq = nc.dram_tensor("dq", [R, QP], FP8, kind="ExternalOutput")  # SCALE_OUT*d

    with tile.TileContext(nc) as tc:
        with (
            tc.tile_pool(name="const", bufs=1) as const,
            tc.tile_pool(name="wdpool", bufs=16) as wdpool,
            tc.tile_pool(name="opool", bufs=3) as opool,
            tc.tile_pool(name="pspool", bufs=8, space="PSUM") as pspool,
        ):
            # scalar ring: h3 first (tiny, must land before the first matmul;
            # emitted before the dummy activation so the ACT_TABLE_LOAD does
            # not delay the trigger). Split so the j=0 slice lands first.
            h3t = const.tile([128, 8, R], FP8, name="h3t")
            nc.scalar.dma_start(h3t[:, 0:2, :], h3q[:, 0:2, :])
            nc.scalar.dma_start(h3t[:, 2:8, :], h3q[:, 2:8, :])
            # sync ring: the 16 wd chunks, streaming; chunk 0 split into
            # ko-pair pieces so the first matmul only waits 128KB
            wdt = []
            for c in range(16):
                t = wdpool.tile([128, 8, 512], FP8, name="wdt")
                if c == 0:
                    for j in range(4):
                        nc.sync.dma_start(
                            t[:, 2 * j : 2 * j + 2, :], wdq[0, :, 2 * j : 2 * j + 2, :]
                        )
                else:
                    nc.sync.dma_start(t[:], wdq[c])
                wdt.append(t)
            scratch = const.tile([1, 2], F32, name="scratch")
            nc.vector.memset(scratch[:], 0.0)
            # dummy activation: hoists ACT_TABLE_LOAD off the critical path
            nc.scalar.activation(
                scratch[:, 1:2],
                scratch[:, 0:1],
                mybir.ActivationFunctionType.Relu,
                bias=scratch[:, 0:1],
            )
            # PE warmup: ~3.5us of back-to-back dummy matmuls on scratch data
            # while the first wd/h3 DMAs are in flight, so the HAM clock-gate
            # is at 8/8 (2.4 GHz) when the real stream starts (cold matmuls
            # run at 1.2 GHz for the first ~3.4us otherwise)
            wsc = const.tile([128, 2, 512], FP8, name="wsc")
            nc.vector.memset(wsc[:], 0.0)
            wps = pspool.tile([128, 512], F32, name="ps")
            warm_mm = None
            for k in range(9):
                mm = nc.tensor.matmul(
                    wps[:],
                    wsc[:, :, 0:128],
                    wsc[:],
                    start=True,
                    stop=True,
                    perf_mode=DR,
                )
                if warm_mm is not None:
                    add_dep_helper(mm.ins, warm_mm.ins, sync=False, reason="pe-order")
                warm_mm = mm

            # big layer: dq[r, c] = SCALE_OUT * h3.T @ wd, DoubleRow fp8,
            # chunk-major so the PE chases the wd DMA stream
            dq_t = dq.rearrange("(s p) c -> p s c", p=128)  # [128, NSLOTS, QP]
            OSC = SCALE_OUT / SCALE  # psum -> out rescale (exact power of 2)
            # output windows (chunk counts): smaller final windows so the
            # last window's DMA tail is short
            WINDOWS = [4, 4, 4, 2, 2]
            prev_mm = warm_mm
            ot = None
            ev = 0
            wbase = 0
            wi = 0
            for c in range(16):
                if ot is None:
                    ot = opool.tile([128, NSLOTS, WINDOWS[wi] * 512], FP8, name="ot")
                for slot in range(NSLOTS):
                    pt = pspool.tile([128, 512], F32, name="ps")
                    for j in range(4):
                        mm = nc.tensor.matmul(
                            pt[:],
                            h3t[:, 2 * j : 2 * j + 2, slot * 128 : (slot + 1) * 128],
                            wdt[c][:, 2 * j : 2 * j + 2, :],
                            start=(j == 0),
                            stop=(j == 3),
                            perf_mode=DR,
                        )
                        if prev_mm is not None:
                            add_dep_helper(
                                mm.ins, prev_mm.ins, sync=False, reason="pe-order"
                            )
                        prev_mm = mm
                    osl = slice((c - wbase) * 512, (c - wbase + 1) * 512)
                    if ev % 2 == 0:
                        nc.scalar.mul(ot[:, slot, osl], pt[:], OSC)
                    else:
                        nc.vector.tensor_scalar_mul(ot[:, slot, osl], pt[:], OSC)
                    ev += 1
                if c - wbase + 1 == WINDOWS[wi]:
                    csl = slice(wbase * 512, (c + 1) * 512)
                    if wi >= 3:
                        # final windows: per-slot pieces on the (now idle)
                        # sync ring so the tail DMA starts right after each
                        # slot's last evacuation
                        for slot in range(NSLOTS):
                            nc.sync.dma_start(
                                dq_t[:, slot, csl], ot[:, slot, :]
                            )
                    else:
                        nc.scalar.dma_start(dq_t[:, :, csl], ot[:])
                    ot = None
                    wbase = c + 1
                    wi += 1
    nc.compile()
    return nc


def _ensure_ntff_hook():
    """Provide antenv.axon_hooks (absent in this image) so trace=True works."""
    import sys
    import types

    try:
        from antenv.axon_hooks import get_axon_ntff_profile_hook  # noqa: F401

        return
    except ImportError:
        pass
    try:
        import antenv
        from trn_agent_boot.trn_boot import _ntff_profile_via_ctypes

        hook = _ntff_profile_via_ctypes("/opt/axon/libaxon_pjrt.so")
        mod = types.ModuleType("antenv.axon_hooks")
        mod._hook = hook
        mod.set_axon_ntff_profile_hook = lambda h: setattr(mod, "_hook", h)
        mod.get_axon_ntff_profile_hook = lambda: mod._hook
        sys.modules["antenv.axon_hooks"] = mod
        antenv.axon_hooks = mod
    except Exception:
        pass


def _h3_f32(x, ws, bs):
    h = x.astype(np.float32)
    for i in range(3):
        h = np.maximum(h @ ws[i] + bs[i], np.float32(0))
    return h


def _exact_h3(x, ws, bs):
    h = x.astype(np.float64)
    for i in range(3):
        h = np.maximum(h @ ws[i].astype(np.float64) + bs[i].astype(np.float64), 0)
    return h


def kernel(**inputs) -> np.ndarray:
    global last_results
    x = np.ascontiguousarray(inputs["x"], dtype=np.float32)
    gumbel = np.ascontiguousarray(inputs["gumbel"], dtype=np.float32)
    bw = [np.asarray(inputs[f"bw{i}"], dtype=np.float32) for i in range(4)]
    bb = [np.asarray(inputs[f"bb{i}"], dtype=np.float32) for i in range(4)]
    sw = [np.asarray(inputs[f"sw{i}"], dtype=np.float32) for i in range(4)]
    sb = [np.asarray(inputs[f"sb{i}"], dtype=np.float32) for i in range(4)]

    mask_big = x[:, 0] > 0.0
    b = int(mask_big.sum())
    # stable sort: big rows first, original order within groups
    perm = np.argsort(~mask_big, kind="stable")
    x_sorted = x[perm]

    def wd_of(w3):
        wd = w3[:, 0::2] - w3[:, 1::2]
        # pad each 8160-col quarter independently to 8192 cols
        wdp = np.zeros((1024, QP * 4), dtype=np.float32)
        for q in range(4):
            wdp[:, q * QP : q * QP + QCOLS] = wd[:, q * QCOLS : (q + 1) * QCOLS]
        wdp *= ALPHA
        np.clip(wdp, -240.0, 240.0, out=wdp)
        return wdp.astype(NP_FP8)

    def wd_pack(quarter):
        # [1024, QP] -> [16, 128, 8, 512]: chunk-major, contiguous per chunk
        # (device reads 4KB/partition contiguous instead of 512B strided runs)
        a = quarter.reshape(8, 128, 16, 512)  # [ko, p, c, n]
        return np.ascontiguousarray(a.transpose(2, 1, 0, 3))

    wd8 = {"big": wd_of(bw[3]), "small": wd_of(sw[3])}
    wd_f32 = {
        "big": bw[3][:, 0::2] - bw[3][:, 1::2],
        "small": sw[3][:, 0::2] - sw[3][:, 1::2],
    }
    bdd = {"big": bb[3][0::2] - bb[3][1::2], "small": sb[3][0::2] - sb[3][1::2]}
    mlp_w = {"big": bw[:3], "small": sw[:3]}
    mlp_b = {"big": bb[:3], "small": sb[:3]}

    # whole-chunk expert assignment over sorted rows: "first s chunks big".
    # s forced even so both groups have exactly 2 chunks; misrouted rows
    # (true expert != chunk expert) are recomputed exactly on host below.
    mis_by_s = {0: b, 2: abs(b - 256), 4: B - b}
    s = min(mis_by_s, key=mis_by_s.get)
    groups = [
        ("big" if 0 < s else "small", [0, 1]),
        ("big" if 2 < s else "small", [2, 3]),
    ]
    assigned_big = np.zeros(B, dtype=bool)
    assigned_big[: s * 128] = True
    true_big = np.arange(B) < b  # in sorted order
    mis_sorted = np.nonzero(assigned_big != true_big)[0]

    if "p" not in _program_cache:
        _program_cache["p"] = build_program()
    nc = _program_cache["p"]

    # host h3 per group (exact fp32 MLP on the group's 256 sorted rows)
    def h3q_pack(rows, exp):
        h3 = _h3_f32(x_sorted[rows], mlp_w[exp], mlp_b[exp])  # [R, 1024]
        h3 = np.clip(h3 * BETA, 0, 240.0)
        # h3q[p, m, r] = BETA*h3[r, 128m + p]
        return np.ascontiguousarray(
            (h3.T).reshape(8, 128, R).transpose(1, 0, 2)
        ).astype(NP_FP8)

    in_maps = []
    for g, (exp, chunks) in enumerate(groups):
        rows = np.arange(chunks[0] * 128, (chunks[-1] + 1) * 128)
        h3q = h3q_pack(rows, exp)
        for q in range(4):
            qsl = slice(q * QP, (q + 1) * QP)
            in_maps.append({"h3q": h3q, "wdq": wd_pack(wd8[exp][:, qsl])})

    trace = bool(int(os.environ.get("CC_KERNEL_TRACE", "0")))
    if trace:
        _ensure_ntff_hook()
    try:
        res = run_bass_kernel_spmd(
            nc,
            in_maps,
            core_ids=list(range(NCORES)),
            trace=trace,
            trace_cores=list(range(NCORES)) if trace else None,
        )
    except Exception:
        if not trace:
            raise
        res = run_bass_kernel_spmd(nc, in_maps, core_ids=list(range(NCORES)))
    last_results = res

    # ---- assemble d (unscaled) in sorted row order ----
    d_sorted = np.empty((B, E), dtype=np.float32)
    for g in range(2):
        r0 = g * 256
        for q in range(4):
            shard = res.results[g * 4 + q]["dq"]
            d_sorted[r0 : r0 + 256, q * QCOLS : (q + 1) * QCOLS] = (
                shard[:, :QCOLS].astype(np.float32) / SCALE_OUT
            )

    # exact d for misrouted rows (host BLAS, true expert)
    if mis_sorted.size:
        for exp in ("big", "small"):
            selm = true_big[mis_sorted] == (exp == "big")
            if not selm.any():
                continue
            rws = mis_sorted[selm]
            h3e = _h3_f32(x_sorted[rws], mlp_w[exp], mlp_b[exp])
            d_sorted[rws] = h3e @ wd_f32[exp]

    # unsort rows
    d_full = np.empty_like(d_sorted)
    d_full[perm] = d_sorted
    global last_d_full
    last_d_full = d_full

    # exact gd and per-row bdd; margins
    bdd_sel = np.where(mask_big[:, None], bdd["big"][None, :], bdd["small"][None, :])
    gd = gumbel[:, :, 1].astype(np.float32) - gumbel[:, :, 0].astype(np.float32)
    t_full = d_full + bdd_sel - gd
    dec_full = t_full >= 0.0

    # ---- exact patch of near-tie edges ----
    thr = TOL_ABS + TOL_REL * np.abs(d_full)
    near_r, near_c = np.nonzero(np.abs(t_full) < thr)
    if near_r.size:
        gde = (
            gumbel[near_r, near_c, 1].astype(np.float64)
            - gumbel[near_r, near_c, 0].astype(np.float64)
        )
        for exp, msk in (("big", mask_big), ("small", ~mask_big)):
            selp = msk[near_r]
            if not selp.any():
                continue
            r, c = near_r[selp], near_c[selp]
            ws = mlp_w[exp]
            bs = mlp_b[exp]
            h3e = _exact_h3(x, ws, bs)  # [B, 1024] float64
            d = np.einsum("ij,ji->i", h3e[r], wd_f32[exp][:, c].astype(np.float64))
            m = d + bdd[exp][c] - gde[selp]
            dec_full[r, c] = m >= 0
    dec_full = dec_full.astype(np.float32)

    # ---- scatter to symmetric adjacency ----
    iu, ju = np.triu_indices(N_NODES, k=1)
    flat_idx = iu * N_NODES + ju
    out = np.zeros((B, N_NODES * N_NODES), dtype=np.float32)
    out[:, flat_idx] = dec_full
    out = out.reshape(B, N_NODES, N_NODES)
    out = out + np.swapaxes(out, 1, 2)
    return out
